# revision 19
# baseline (speedup 1.0000x reference)
import numpy as np
from contextlib import ExitStack

import concourse.bass as bass
import concourse.tile as tile
from concourse import bacc, mybir

# problem constants (hardcoded per contract)
N = 4096          # points
C = 20            # feature channels
K = 6             # boxes
M = 3             # views
G = K * M         # 18 groups
RES = 48          # H = W
NCORES = 8
SROWS = RES // NCORES          # 6 grid rows per core
SLOC = SROWS * RES             # 288 cells per core
NSAMPLE = 16
RADIUS2 = 9.0

TRACE = False
_last = {}

_f32 = mybir.dt.float32
_bf16 = mybir.dt.bfloat16
_ALU = mybir.AluOpType
_ACT = mybir.ActivationFunctionType
BF16 = mybir.dt.np(_bf16)

# x-slab half-width: a point can only be in-ball for a cell row gx when
# |x - gx| < RADIUS, so core c (rows 6c..6c+5) only needs x in (6c-3, 6c+8)
SLAB_LO = -3.0
SLAB_HI = float(SROWS) + 5.0


def _build_nc(cappts):
    """Per-group-capped ball query + first-16 aggregation.

    Inputs per core (slab-filtered points, group-major, exact per-group
    point caps):
      AXX/AXY [1, sum(cappts)] f32  point coords (pad: 1e6)
      PD   [128, sum(nblocks)] bf16  per-block point scores s1-s0
      ONES [1, sum(cappts)] f32  constant-term row for the score matmul
      B4   [4, SLOC] f32  cell polynomials [2sx, 2sy, -1, R^2-sx^2-sy^2]
      TRIB/FIXB [128,128] bf16  prefix-scan / full-sum upgrade matrices
    x^2+y^2 is computed on device (IEEE fp32, bit-identical to host).
    Output: OUT [G, SLOC] bf16.
    """
    nblks = [-(-int(w) // 128) for w in cappts]
    BTOT = int(sum(nblks))
    PTOT = int(sum(cappts))
    nc = bacc.Bacc("TRN2", target_bir_lowering=False, debug=False,
                   num_devices=NCORES)
    AXX = nc.dram_tensor("AXX", [1, PTOT], _f32, kind="ExternalInput").ap()
    AXY = nc.dram_tensor("AXY", [1, PTOT], _f32, kind="ExternalInput").ap()
    PD = nc.dram_tensor("PD", [128, BTOT], _bf16, kind="ExternalInput").ap()
    ONES = nc.dram_tensor("ONES", [1, PTOT], _f32, kind="ExternalInput").ap()
    B4 = nc.dram_tensor("B4", [4, SLOC], _f32, kind="ExternalInput").ap()
    TRIB = nc.dram_tensor("TRIB", [128, 128], _bf16, kind="ExternalInput").ap()
    FIXB = nc.dram_tensor("FIXB", [128, 128], _bf16, kind="ExternalInput").ap()
    OUT = nc.dram_tensor("OUT", [G, SLOC], _bf16, kind="ExternalOutput").ap()

    with ExitStack() as ctx:
        tc = ctx.enter_context(tile.TileContext(nc))
        consts = ctx.enter_context(tc.tile_pool(name="consts", bufs=1))
        ppool = ctx.enter_context(tc.tile_pool(name="ppool", bufs=2))
        wpool = ctx.enter_context(tc.tile_pool(name="wpool", bufs=4))
        spool = ctx.enter_context(tc.tile_pool(name="spool", bufs=4))
        fin = ctx.enter_context(tc.tile_pool(name="fin", bufs=1))
        rowpool = ctx.enter_context(tc.tile_pool(name="rowpool", bufs=2))
        scps = ctx.enter_context(
            tc.tile_pool(name="scps", bufs=3, space=bass.MemorySpace.PSUM))
        ups = ctx.enter_context(
            tc.tile_pool(name="ups", bufs=2, space=bass.MemorySpace.PSUM))
        stps = ctx.enter_context(
            tc.tile_pool(name="stps", bufs=2, space=bass.MemorySpace.PSUM))

        b4_t = consts.tile([4, SLOC], _f32)
        nc.sync.dma_start(b4_t[:], B4)
        tri_t = consts.tile([128, 128], _bf16)
        nc.sync.dma_start(tri_t[:], TRIB)
        fix_t = consts.tile([128, 128], _bf16)
        nc.sync.dma_start(fix_t[:], FIXB)

        # assemble the score stationary [x, y, x^2+y^2, 1] once for all
        # groups; compute engines need 32-aligned partition bases, so y is
        # squared in a base-0 scratch tile and rows land in big_t via DMA
        big_t = consts.tile([4, PTOT], _f32)
        nc.sync.dma_start(big_t[0:1, :], AXX)
        yt = consts.tile([1, PTOT], _f32)
        nc.sync.dma_start(yt[:], AXY)
        nc.sync.dma_start(big_t[1:2, :], yt[:])
        nc.sync.dma_start(big_t[3:4, :], ONES)
        sq_t = consts.tile([1, PTOT], _f32)
        nc.vector.tensor_tensor(sq_t[:], big_t[0:1, :], big_t[0:1, :],
                                _ALU.mult)
        ysq_t = consts.tile([1, PTOT], _f32)
        nc.vector.tensor_tensor(ysq_t[:], yt[:], yt[:], _ALU.mult)
        nc.vector.tensor_tensor(sq_t[:], sq_t[:], ysq_t[:], _ALU.add)
        nc.sync.dma_start(big_t[2:3, :], sq_t[:])

        d_all = fin.tile([G, SLOC], _f32, tag="d_all")
        c_all = fin.tile([G, SLOC], _f32, tag="c_all")

        poff = 0
        boff = 0
        for g in range(G):
            w_g = int(cappts[g])
            nb = nblks[g]
            p_t = ppool.tile([128, 2 * nb], _bf16, tag="p")
            nc.sync.dma_start(p_t[:, 0::2], PD[:, boff:boff + nb])
            nc.vector.memset(p_t[:, 1::2], 1.0)
            u_ps = ups.tile([128, SLOC], _f32, tag="u")
            st_ps = stps.tile([2, SLOC], _f32, tag="st")
            for b in range(nb):
                wb = min(128, w_g - 128 * b)
                score_ps = scps.tile([128, SLOC], _f32, tag="sc")
                nc.tensor.matmul(score_ps[0:wb, :],
                                 big_t[:, poff + 128 * b:poff + 128 * b + wb],
                                 b4_t[:], start=True, stop=True)
                within = wpool.tile([128, SLOC], _bf16, tag="w")
                nc.vector.tensor_scalar(within[0:wb, :], score_ps[0:wb, :],
                                        0.0, None, _ALU.is_gt)
                # u = (total within count of prior blocks) + excl prefix
                #     - 16*within; sel = u < 0 picks the first 16 in-ball
                nc.tensor.matmul(u_ps[0:wb, :], tri_t[0:wb, 0:wb],
                                 within[0:wb, :],
                                 start=(b == 0), stop=(b == nb - 1))
                sel = spool.tile([128, SLOC], _bf16, tag="s")
                nc.vector.tensor_scalar(sel[0:wb, :], u_ps[0:wb, :], 0.0,
                                        None, _ALU.is_lt)
                if b < nb - 1:
                    # upgrade this block's TRI contribution to its full
                    # within-count so u carries across blocks
                    nc.tensor.matmul(u_ps[:], fix_t[0:wb, :], within[0:wb, :],
                                     start=False, stop=False)
                # state rows: [sum(s1-s0), cnt] over selected points
                nc.tensor.matmul(st_ps[:], p_t[0:wb, 2 * b:2 * (b + 1)],
                                 sel[0:wb, :],
                                 start=(b == 0), stop=(b == nb - 1))
            # bounce the state rows through SBUF (ACT at partition 0), then
            # scatter to per-group partitions with DMA (no alignment limits)
            tmp = rowpool.tile([2, SLOC], _f32, tag="tmp")
            nc.scalar.activation(tmp[:], st_ps[:], _ACT.Copy)
            nc.sync.dma_start(d_all[g:g + 1, :], tmp[0:1, :])
            nc.sync.dma_start(c_all[g:g + 1, :], tmp[1:2, :])
            poff += w_g
            boff += nb

        # finalize all groups at once:
        # out = (cnt>0) * sigmoid(sum(s1-s0)/max(cnt,1)) * 255
        cntc = fin.tile([G, SLOC], _f32, tag="cntc")
        nc.vector.tensor_scalar(cntc[:], c_all[:], 1.0, None, _ALU.max)
        rcp = fin.tile([G, SLOC], _f32, tag="rcp")
        nc.vector.reciprocal(rcp[:], cntc[:])
        nfd = fin.tile([G, SLOC], _f32, tag="nfd")
        nc.vector.tensor_tensor(nfd[:], d_all[:], rcp[:], _ALU.mult)
        sig = fin.tile([G, SLOC], _f32, tag="sig")
        nc.scalar.activation(sig[:], nfd[:], _ACT.Sigmoid)
        gate = fin.tile([G, SLOC], _f32, tag="gate")
        nc.vector.tensor_scalar(gate[:], c_all[:], 0.5, 255.0,
                                _ALU.is_gt, _ALU.mult)
        orow = fin.tile([G, SLOC], _bf16, tag="orow")
        nc.vector.tensor_tensor(orow[:], sig[:], gate[:], _ALU.mult)
        nc.sync.dma_start(OUT, orow[:])
    nc.compile()
    return nc


# ---------------------------------------------------------------------------
# Cached SPMD dispatch. run_bass_kernel_spmd rebuilds its jax.jit closure on
# every invocation, which forces a full XLA retrace+recompile (~0.9s) per
# call; the NEFF itself is unchanged between calls. Build the jitted
# shard_map executable once and reuse it, fetch the output with a single
# host transfer, and keep value-independent inputs resident on device.
# ---------------------------------------------------------------------------

_CACHE = {}
_CONST_NAMES = {"ONES", "B4", "TRIB", "FIXB"}
_PAD = np.float32(1e6)


def _build_dispatch(nc):
    import jax
    from jax.experimental.shard_map import shard_map
    from jax.sharding import Mesh, NamedSharding, PartitionSpec
    from concourse.bass2jax import (
        _bass_exec_p, install_neuronx_cc_hook, partition_id_tensor)

    install_neuronx_cc_hook()
    assert nc.dbg_addr is None

    partition_name = nc.partition_id_tensor.name if nc.partition_id_tensor else None
    in_names, out_names, out_avals, zero_shapes = [], [], [], []
    for alloc in nc.m.functions[0].allocations:
        if not isinstance(alloc, mybir.MemoryLocationSet):
            continue
        name = alloc.memorylocations[0].name
        if alloc.kind == "ExternalInput":
            if name != partition_name:
                in_names.append(name)
        elif alloc.kind == "ExternalOutput":
            shape = tuple(alloc.tensor_shape)
            dtype = mybir.dt.np(alloc.dtype)
            out_names.append(name)
            out_avals.append(jax.core.ShapedArray(shape, dtype))
            zero_shapes.append((shape, dtype))
    n_params = len(in_names)
    bind_names = in_names + out_names
    if partition_name is not None:
        bind_names.append(partition_name)

    def _body(*args):
        operands = list(args)
        if partition_name is not None:
            operands.append(partition_id_tensor())
        outs = _bass_exec_p.bind(
            *operands,
            out_avals=tuple(out_avals),
            in_names=tuple(bind_names),
            out_names=tuple(out_names),
            lowering_input_output_aliases=(),
            sim_require_finite=True,
            sim_require_nnan=True,
            nc=nc,
        )
        return tuple(outs)

    devices = jax.devices()[:NCORES]
    assert len(devices) == NCORES
    mesh = Mesh(np.asarray(devices), ("core",))
    n_outs = len(out_names)
    donate = tuple(range(n_params, n_params + n_outs))
    in_specs = (PartitionSpec("core"),) * (n_params + n_outs)
    out_specs = (PartitionSpec("core"),) * n_outs
    sharded = jax.jit(
        shard_map(_body, mesh=mesh, in_specs=in_specs, out_specs=out_specs,
                  check_rep=False),
        donate_argnums=donate,
        keep_unused=True,
    )
    sharding = NamedSharding(mesh, PartitionSpec("core"))

    def put(x):
        return jax.device_put(x, sharding)

    return {
        "fn": sharded, "in_names": in_names, "out_names": out_names,
        "zero_shapes": zero_shapes, "put": put,
    }


def _dispatch(in_maps, const_map):
    """in_maps: per-core dict of value-dependent arrays. const_map: dict of
    per-core-stacked value-independent arrays, committed to device once."""
    d = _CACHE["disp"]
    if "const_cache" not in d:
        d["const_cache"] = {name: d["put"](arr)
                            for name, arr in const_map.items()}
    concat_in = []
    for name in d["in_names"]:
        if name in d["const_cache"]:
            concat_in.append(d["const_cache"][name])
            continue
        arr = np.concatenate([m[name] for m in in_maps], axis=0)
        concat_in.append(arr)
    concat_zeros = [np.zeros((NCORES * s[0], *s[1:]), dt)
                    for s, dt in d["zero_shapes"]]
    out_arrs = d["fn"](*concat_in, *concat_zeros)
    return {name: np.asarray(out_arrs[i]) for i, name in enumerate(d["out_names"])}


def _build_consts(cappts):
    PTOT = int(sum(cappts))
    gx, gy = np.meshgrid(np.arange(RES), np.arange(RES), indexing='ij')
    samples = np.stack([gx, gy], -1).reshape(-1, 2).astype(np.float32)
    TRIc = np.triu(np.ones((128, 128), np.float32), 1)
    np.fill_diagonal(TRIc, -float(NSAMPLE))
    FIXc = np.ones((128, 128), np.float32) - TRIc
    onesr = np.ones((1, PTOT), np.float32)
    b4s, oness, tris, fixs = [], [], [], []
    for cidx in range(NCORES):
        s = samples[cidx * SLOC:(cidx + 1) * SLOC]
        b4s.append(np.stack([
            2.0 * s[:, 0], 2.0 * s[:, 1],
            -np.ones(SLOC, np.float32),
            RADIUS2 - (s[:, 0] ** 2 + s[:, 1] ** 2),
        ]).astype(np.float32))
        oness.append(onesr)
        tris.append(TRIc.astype(BF16))
        fixs.append(FIXc.astype(BF16))
    return {
        "B4": np.concatenate(b4s, axis=0),
        "ONES": np.concatenate(oness, axis=0),
        "TRIB": np.concatenate(tris, axis=0),
        "FIXB": np.concatenate(fixs, axis=0),
    }


def kernel(xyz, features, boxes, theta, phi, res):
    xyz = np.asarray(xyz, np.float32)[0]        # (N,3)
    features = np.asarray(features, np.float32)[0]  # (N,C)
    boxes = np.asarray(boxes, np.float32)[0]    # (K,6)
    theta = np.asarray(theta, np.float32)
    phi = np.asarray(phi, np.float32)
    res = int(res)
    H = W = res

    # ---- host prep: projection + per-group normalization (identical
    # arithmetic to the reference so the fp32 ball-query boundary decisions
    # match), then slab-filter points per (group, core)
    sint, cost = np.sin(theta), np.cos(theta)
    sinp, cosp = np.sin(phi), np.cos(phi)
    U = np.stack([-sint, cost, np.zeros_like(theta)], -1)
    V = np.stack([cost * sinp, sint * sinp, cosp], -1)
    basis = np.stack([U, V], -1).astype(np.float32)          # (M,3,2)
    center3 = np.stack([cost * cosp, sint * cosp, sinp], -1).astype(np.float32)
    coords_mv = np.einsum('mnd,mdk->mnk',
                          (xyz[None] - center3[:, None]).astype(np.float32),
                          basis).astype(np.float32)          # (M,N,2)
    valid = (np.all(xyz[None] <= boxes[:, None, 3:], -1)
             & np.all(xyz[None] >= boxes[:, None, :3], -1))  # (K,N)
    pts = np.sort(features, -1)[:, -2:].astype(np.float32)   # (N,2)
    dfull = (pts[:, 1] - pts[:, 0]).astype(np.float32)       # (N,)
    p2 = np.array([H, W], np.float32)

    # vectorized per-(box,view) normalization; min/max over the valid subset
    # equals the masked min/max exactly, and the elementwise chain below is
    # the same fp32 op sequence as the reference
    vm4 = valid[:, None, :, None]                            # (K,1,N,1)
    cm = np.broadcast_to(coords_mv[None], (K, M, N, 2))
    cmax = np.where(vm4, cm, -np.inf).max(2)                 # (K,M,2)
    cmin = np.where(vm4, cm, np.inf).min(2)
    ctr = ((cmax + cmin) / 2).astype(np.float32)
    scale = (np.maximum(cmax - cmin, np.float32(1e-5)) / 2).astype(np.float32)
    cn = (((cm - ctr[:, :, None]) / scale[:, :, None] + np.float32(1.0))
          * np.float32(0.8) * p2 / 2 + np.float32(0.1) * p2).astype(np.float32)
    cn = np.where(vm4, cn, np.float32(1e6)).reshape(G, N, 2)
    xflat = np.ascontiguousarray(cn[..., 0]).reshape(G * N)
    yflat = np.ascontiguousarray(cn[..., 1]).reshape(G * N)

    # slab masks and per-group point caps (max over cores, 8-aligned)
    lo = np.arange(NCORES, dtype=np.float32) * SROWS + SLAB_LO   # (NCORES,)
    hi = np.arange(NCORES, dtype=np.float32) * SROWS + SLAB_HI
    xg = cn[..., 0]                                              # (G,N)
    masks = (xg[:, None, :] > lo[None, :, None]) & \
            (xg[:, None, :] < hi[None, :, None])                 # (G,NCORES,N)
    counts = masks.sum(-1)                                       # (G,NCORES)
    cappts = tuple(int(x) for x in
                   np.maximum(8, -(-counts.max(1) // 8) * 8))
    nblks = [-(-w // 128) for w in cappts]
    BTOT = int(sum(nblks))
    PTOT = int(sum(cappts))
    poffs = np.concatenate([[0], np.cumsum(cappts)])
    boffs = np.concatenate([[0], np.cumsum(nblks)])

    if _CACHE.get("cappts") != cappts:
        _CACHE.clear()
        _CACHE["cappts"] = cappts
        _CACHE["nc"] = _build_nc(cappts)
        _CACHE["disp"] = _build_dispatch(_CACHE["nc"])
        _CACHE["consts"] = _build_consts(cappts)

    in_maps = []
    for cidx in range(NCORES):
        AXXc = np.full((1, PTOT), _PAD, np.float32)
        AXYc = np.full((1, PTOT), _PAD, np.float32)
        PDc = np.zeros((128, BTOT), BF16)
        idxs = [np.nonzero(masks[g, cidx])[0] for g in range(G)]
        src = np.concatenate([g * N + idxs[g] for g in range(G)])
        q = np.concatenate([np.arange(idxs[g].size) for g in range(G)])
        gid = np.concatenate([np.full(idxs[g].size, g) for g in range(G)])
        dst = poffs[gid] + q
        AXXc[0, dst] = xflat[src]
        AXYc[0, dst] = yflat[src]
        PDc[q % 128, boffs[gid] + q // 128] = dfull[src % N]
        in_maps.append({"AXX": AXXc, "AXY": AXYc, "PD": PDc})

    results = _dispatch(in_maps, _CACHE["consts"])
    _last['exec_time_ns'] = None
    out_g = results["OUT"].reshape(NCORES, G, SROWS, W).astype(np.float32)
    full = np.concatenate([out_g[c] for c in range(NCORES)], axis=1)  # (G,H,W)
    out = np.broadcast_to(full[:, None, :, :], (G, 3, H, W)).astype(np.float32)
    return np.ascontiguousarray(out)
